# revision 17
# baseline (speedup 1.0000x reference)
"""Trainium2 Bass kernel for nn_ASMLoc_Base (topk_masking).

B=32,T=1024,D=2048,NCLS=21. Data-parallel over batch: 4 videos per core x 8 cores.
Per core:
  prep:   gpsimd cast-DMAs f32->bf16 (DRAM->DRAM), xbar DMA-transposes to build
          W^T [din,dout] (SBUF, per dout-half) and x^T [din,t] (DRAM, padded).
  conv:   emb[dout,t] = relu(sum_k W_k^T.T @ x^T(shift k) + b): 48 accumulating
          bf16 128x128x512 matmuls per psum tile, ReLU+bias on ScalarE.
  stage2: Z[24,t] = cmb @ emb (rows 0..20 cls, 21/22 att, 23 att-diff), PE
          transposes -> per-t softmaxes (cas/fg/bg, ScalarE exp only) + temp_att;
          fg/bg cas packed per video [42, t] for top-k.
  topk:   per-video binary search (24 iters) for the k-th largest, masked sum,
          mean, then per-video softmax over classes -> fg_cls/bg_cls.
Emission order interleaves weight/input prep with conv so PE starts early.
"""

import os
import sys

for _p in ("/opt/trn_rl_repo", "/root/.axon_site/_ro/trn_rl_repo"):
    if os.path.isdir(_p) and _p not in sys.path:
        sys.path.insert(0, _p)

import numpy as np

import concourse.bass as bass
import concourse.tile as tile
from concourse import bacc, mybir
from concourse.bass_utils import run_bass_kernel_spmd

B, T, D, NCLS = 32, 1024, 2048, 21
NCORES = 8
BL = B // NCORES            # videos per core
FGK, BGK = T // 8, T // 3   # 128, 341
NDT = D // 128              # 16 din (and dout) tiles
NCR = NCLS + 3              # 21 cls + att0 + att1 + attdiff = 24
F32 = mybir.dt.float32
BF16 = mybir.dt.bfloat16
AX = mybir.AxisListType
OP = mybir.AluOpType
AF = mybir.ActivationFunctionType

N_ITER = 24  # topk binary-search iterations


def build_nc():
    nc = bacc.Bacc("TRN2", target_bir_lowering=False, debug=False)

    xi = nc.declare_dram_parameter("x", [BL, T, D], F32, isOutput=False)
    cw = nc.declare_dram_parameter("conv_w", [D, D, 3], F32, isOutput=False)
    cbias = nc.declare_dram_parameter("cbias", [128, NDT], F32, isOutput=False)
    cmbt = nc.declare_dram_parameter("cmbt", [D, NCR], F32, isOutput=False)
    zbias = nc.declare_dram_parameter("zbias", [NCR, 1], F32, isOutput=False)
    ident = nc.declare_dram_parameter("ident", [128, 128], F32, isOutput=False)
    kvec = nc.declare_dram_parameter("kvec", [42, 1], F32, isOutput=False)
    kinv = nc.declare_dram_parameter("kinv", [42, 1], F32, isOutput=False)

    o_fg_cls = nc.declare_dram_parameter("fg_cls", [BL, NCLS], F32, isOutput=True)
    o_bg_cls = nc.declare_dram_parameter("bg_cls", [BL, NCLS], F32, isOutput=True)
    o_ta = nc.declare_dram_parameter("temp_att", [BL, T, 2], F32, isOutput=True)
    o_cas = nc.declare_dram_parameter("cas_sm", [BL, T, NCLS], F32, isOutput=True)
    o_fg = nc.declare_dram_parameter("fg_sm", [BL, T, NCLS], F32, isOutput=True)
    o_bg = nc.declare_dram_parameter("bg_sm", [BL, T, NCLS], F32, isOutput=True)

    w_bf = nc.dram_tensor("w_bf", [3, D, D], BF16)     # cast + k-deinterleaved
    x_bf = nc.dram_tensor("x_bf", [BL, T, D], BF16)
    emb_bf = nc.dram_tensor("emb_bf", [BL, D, T], BF16)

    with tile.TileContext(nc) as tc, \
         tc.tile_pool(name="persist", bufs=1) as persist, \
         tc.tile_pool(name="psZ", bufs=1, space="PSUM") as psZ, \
         tc.tile_pool(name="psZT", bufs=1, space="PSUM") as psZT, \
         tc.tile_pool(name="psBC", bufs=1, space="PSUM") as psBC:

        # ---------------- persistent small tiles ----------------
        cb_sb = persist.tile([128, NDT], F32)
        nc.gpsimd.dma_start(cb_sb[:], cbias[:])
        zb_sb = persist.tile([NCR, 1], F32)
        nc.gpsimd.dma_start(zb_sb[:], zbias[:])
        ident_f = persist.tile([128, 128], F32)
        nc.gpsimd.dma_start(ident_f[:], ident[:])
        kt_sb = persist.tile([42, 1], F32)
        nc.gpsimd.dma_start(kt_sb[:], kvec[:])
        ki_sb = persist.tile([42, 1], F32)
        nc.gpsimd.dma_start(ki_sb[:], kinv[:])
        ones21 = persist.tile([1, NCLS], F32)
        nc.vector.memset(ones21[:], 1.0)
        # cmbt f32 -> bf16 [128, dt, 24], cast in DMA
        cmbT = persist.tile([128, NDT, NCR], BF16)
        nc.gpsimd.dma_start(cmbT[:], cmbt.rearrange("(a p) c -> p a c", p=128))

        def emit_once(rep):
          with tc.tile_pool(name=f"wf{rep}", bufs=1) as poolWf, \
               tc.tile_pool(name=f"wb{rep}", bufs=2) as poolWb, \
               tc.tile_pool(name=f"xt{rep}", bufs=1) as poolXT, \
               tc.tile_pool(name=f"wT{rep}", bufs=1) as poolWT, \
               tc.tile_pool(name=f"psC{rep}", bufs=3, space="PSUM") as poolPS, \
               tc.tile_pool(name=f"embo{rep}", bufs=4) as poolEmb, \
               tc.tile_pool(name=f"er{rep}", bufs=5) as poolER, \
               tc.tile_pool(name=f"zs{rep}", bufs=2) as poolZS, \
               tc.tile_pool(name=f"s2s{rep}", bufs=2) as poolS, \
               tc.tile_pool(name=f"sm{rep}", bufs=4) as poolSm, \
               tc.tile_pool(name=f"smo{rep}", bufs=6) as poolOut, \
               tc.tile_pool(name=f"tk{rep}", bufs=2) as tk, \
               tc.tile_pool(name=f"tks{rep}", bufs=1) as tks:

            def w_prep_ot(ot):
                # cast-load f32->bf16, k-deinterleave on DVE, store per-k
                wf = poolWf.tile([128, D, 3], BF16, tag="wf")
                nc.gpsimd.dma_start(
                    wf[:], cw[ot * 128:(ot + 1) * 128, :, :])
                for k in range(3):
                    wb = poolWb.tile([128, D], BF16, tag="wb")
                    nc.vector.tensor_copy(wb[:], wf[:, :, k])
                    nc.gpsimd.dma_start(
                        w_bf[k, ot * 128:(ot + 1) * 128, :], wb[:])

            def wT_load8(h8, slot):
                # eighth of the dout dim: [din, 256] per (k, dt)
                wT = poolWT.tile([128, 3 * NDT * 256], BF16, tag=slot)
                for k in range(3):
                    for dt in range(NDT):
                        nc.sync.dma_start(
                            wT[:, (k * NDT + dt) * 256:(k * NDT + dt + 1) * 256],
                            w_bf[k, h8 * 256:(h8 + 1) * 256,
                                 dt * 128:(dt + 1) * 128],
                            transpose=True,
                        )
                return wT

            def xT_load(v, slot):
                # resident x^T [din, t] for one video, zero-padded t edges
                xT = poolXT.tile([128, NDT, T + 2], BF16, tag=slot)
                nc.vector.memset(xT[:, :, 0:1], 0.0)
                nc.vector.memset(xT[:, :, T + 1:T + 2], 0.0)
                for dt in range(NDT):
                    nc.sync.dma_start(
                        xT[:, dt, 1:T + 1],
                        x_bf[v, :, dt * 128:(dt + 1) * 128],
                        transpose=True)
                return xT

            def conv8(h8, v, wT, xT):
                for c in range(2):
                    for ot in range(2):
                        ps = poolPS.tile([128, 512], F32)
                        for dt in range(NDT):
                            for k in range(3):
                                base = (k * NDT + dt) * 256 + ot * 128
                                nc.tensor.matmul(
                                    ps[:],
                                    wT[:, base:base + 128],
                                    xT[:, dt, c * 512 + k:c * 512 + k + 512],
                                    start=(dt == 0 and k == 0),
                                    stop=(dt == NDT - 1 and k == 2),
                                )
                        g = h8 * 2 + ot
                        emb_t = poolEmb.tile([128, 512], BF16, tag="embo")
                        nc.scalar.activation(
                            emb_t[:], ps[:], AF.Relu,
                            bias=cb_sb[:, g:g + 1])
                        nc.gpsimd.dma_start(
                            emb_bf[v, g * 128:(g + 1) * 128,
                                   c * 512:(c + 1) * 512],
                            emb_t[:])

            def stage2(v, pack):
                for c in range(2):
                    pz = psZ.tile([NCR, 512], F32)
                    for og in range(NDT):
                        er = poolER.tile([128, 512], BF16, tag="er")
                        nc.sync.dma_start(
                            er[:],
                            emb_bf[v, og * 128:(og + 1) * 128,
                                   c * 512:(c + 1) * 512])
                        nc.tensor.matmul(
                            pz[:], cmbT[:, og, :], er[:],
                            start=(og == 0), stop=(og == NDT - 1))
                    z_sb = poolZS.tile([NCR, 512], F32)
                    nc.vector.tensor_scalar_add(z_sb[:], pz[:], zb_sb[:, 0:1])

                    # fg/bg att rows via exp (stay in exp_and_others act set):
                    # fg = 1/(1+exp(-z23)), bg = 1/(1+exp(z23))
                    ef = poolS.tile([NCR, 512], F32, tag="e2")
                    eb = poolS.tile([NCR, 512], F32, tag="e2")
                    nc.scalar.activation(ef[:], z_sb[:], AF.Exp, scale=-1.0)
                    nc.scalar.activation(eb[:], z_sb[:], AF.Exp)
                    nc.vector.tensor_scalar_add(ef[:], ef[:], 1.0)
                    nc.vector.tensor_scalar_add(eb[:], eb[:], 1.0)
                    nc.vector.reciprocal(ef[:], ef[:])
                    nc.vector.reciprocal(eb[:], eb[:])
                    s_f = poolS.tile([1, 512], F32, tag="sfg")
                    s_g = poolS.tile([1, 512], F32, tag="sfg")
                    nc.gpsimd.dma_start(s_f[:], ef[23:24, :])
                    nc.gpsimd.dma_start(s_g[:], eb[23:24, :])

                    bc_f = psBC.tile([NCLS, 512], F32)
                    bc_g = psBC.tile([NCLS, 512], F32)
                    nc.tensor.matmul(bc_f[:], ones21[:], s_f[:])
                    nc.tensor.matmul(bc_g[:], ones21[:], s_g[:])

                    fcs = poolS.tile([NCLS, 512], F32, tag="cas_stage")
                    gcs = poolS.tile([NCLS, 512], F32, tag="cas_stage")
                    nc.vector.tensor_mul(fcs[:], z_sb[0:NCLS, :], bc_f[:])
                    nc.vector.tensor_mul(gcs[:], z_sb[0:NCLS, :], bc_g[:])
                    nc.gpsimd.dma_start(
                        pack[0:NCLS, c * 512:(c + 1) * 512], fcs[:])
                    nc.gpsimd.dma_start(
                        pack[NCLS:2 * NCLS, c * 512:(c + 1) * 512], gcs[:])

                    for q in range(4):
                        tq = c * 512 + q * 128
                        pzt = psZT.tile([128, NCR], F32)
                        nc.tensor.transpose(
                            pzt[:], z_sb[:, q * 128:(q + 1) * 128],
                            ident_f[0:NCR, 0:NCR])
                        ta = poolSm.tile([128, 2], F32, tag="ta")
                        tb = poolSm.tile([128, 2], F32, tag="tb")
                        nc.scalar.activation(tb[:, 0:1], pzt[:, 23:24], AF.Exp,
                                             scale=-1.0)
                        nc.scalar.activation(tb[:, 1:2], pzt[:, 23:24], AF.Exp)
                        nc.vector.tensor_scalar_add(tb[:], tb[:], 1.0)
                        nc.vector.reciprocal(ta[:], tb[:])
                        nc.gpsimd.dma_start(o_ta[v, tq:tq + 128, :], ta[:])

                        for scol, dst in ((None, o_cas), (0, o_fg), (1, o_bg)):
                            if scol is None:
                                logits = pzt[:, 0:NCLS]
                            else:
                                lg = poolSm.tile([128, NCLS], F32, tag="lg")
                                nc.vector.tensor_scalar_mul(
                                    lg[:], pzt[:, 0:NCLS],
                                    ta[:, scol:scol + 1])
                                logits = lg[:]
                            nm = poolSm.tile([128, 1], F32, tag="nm")
                            nc.vector.tensor_reduce(
                                nm[:], logits, axis=AX.X, op=OP.max, negate=True)
                            ex = poolSm.tile([128, NCLS], F32, tag="ex")
                            sm = poolSm.tile([128, 1], F32, tag="sm")
                            nc.scalar.activation(ex[:], logits, AF.Exp,
                                                 bias=nm[:, 0:1],
                                                 accum_out=sm[:, 0:1])
                            rc = poolSm.tile([128, 1], F32, tag="rc")
                            nc.vector.reciprocal(rc[:], sm[:])
                            oo = poolOut.tile([128, NCLS], F32, tag="oo")
                            nc.vector.tensor_scalar_mul(oo[:], ex[:], rc[:, 0:1])
                            nc.gpsimd.dma_start(dst[v, tq:tq + 128, :], oo[:])

            def topk(v, pack):
                lo = tk.tile([42, 1], F32, tag="lo")
                hi = tk.tile([42, 1], F32, tag="hi")
                mid = tk.tile([42, 1], F32, tag="mid")
                cnt = tk.tile([42, 1], F32, tag="cnt")
                ge = tk.tile([42, 1], mybir.dt.int32, tag="ge")
                lt = tk.tile([42, 1], mybir.dt.int32, tag="lt")
                nc.vector.tensor_reduce(lo[:], pack[:], axis=AX.X, op=OP.min)
                nc.vector.tensor_reduce(hi[:], pack[:], axis=AX.X, op=OP.max)
                # hi += (hi-lo)*1e-6 + 1e-12 so cnt(x>=hi) < k strictly
                nc.vector.tensor_sub(mid[:], hi[:], lo[:])
                nc.vector.tensor_scalar(
                    out=mid[:], in0=mid[:], scalar1=1e-6, scalar2=1e-12,
                    op0=OP.mult, op1=OP.add)
                nc.vector.tensor_add(hi[:], hi[:], mid[:])
                for it in range(N_ITER):
                    nc.vector.tensor_scalar(
                        out=mid[:], in0=lo[:], scalar1=hi[:, 0:1], scalar2=0.5,
                        op0=OP.add, op1=OP.mult)
                    scr = tks.tile([42, T], F32, tag="scr")
                    nc.vector.tensor_scalar(
                        out=scr[:], in0=pack[:], scalar1=mid[:, 0:1],
                        scalar2=None, op0=OP.is_ge, op1=OP.add,
                        accum_out=cnt[:, 0:1])
                    nc.vector.tensor_scalar(
                        out=ge[:], in0=cnt[:], scalar1=kt_sb[:, 0:1],
                        scalar2=None, op0=OP.is_ge)
                    nc.vector.tensor_scalar(
                        out=lt[:], in0=cnt[:], scalar1=kt_sb[:, 0:1],
                        scalar2=None, op0=OP.is_lt)
                    nc.vector.copy_predicated(lo[:], ge[:], mid[:])
                    nc.vector.copy_predicated(hi[:], lt[:], mid[:])
                scr = tks.tile([42, T], F32, tag="scr")
                nc.vector.tensor_scalar(
                    out=scr[:], in0=pack[:], scalar1=lo[:, 0:1], scalar2=None,
                    op0=OP.is_ge, op1=OP.add, accum_out=cnt[:, 0:1])
                ssum = tk.tile([42, 1], F32, tag="ss")
                scr2 = tks.tile([42, T], F32, tag="scr")
                nc.vector.scalar_tensor_tensor(
                    out=scr2[:], in0=pack[:], scalar=lo[:, 0:1], in1=pack[:],
                    op0=OP.is_ge, op1=OP.mult, accum_out=ssum[:, 0:1])
                # mean = (ssum - (cnt-k)*lo) / k
                nc.vector.tensor_sub(cnt[:], cnt[:], kt_sb[:])
                nc.vector.tensor_mul(cnt[:], cnt[:], lo[:])
                nc.vector.tensor_sub(ssum[:], ssum[:], cnt[:])
                nc.vector.tensor_mul(ssum[:], ssum[:], ki_sb[:])
                # [42,1] -> [2,21]; softmax over classes; rows: fg, bg
                mv = tk.tile([2, NCLS], F32, tag="mv")
                nc.gpsimd.dma_start(mv[:], ssum[:, 0:1])
                nm = tk.tile([2, 1], F32, tag="nm2")
                nc.vector.tensor_reduce(nm[:], mv[:], axis=AX.X, op=OP.max,
                                        negate=True)
                ex = tk.tile([2, NCLS], F32, tag="ex2")
                sm = tk.tile([2, 1], F32, tag="sm2")
                nc.scalar.activation(ex[:], mv[:], AF.Exp, bias=nm[:, 0:1],
                                     accum_out=sm[:, 0:1])
                rc = tk.tile([2, 1], F32, tag="rc2")
                nc.vector.reciprocal(rc[:], sm[:])
                oo = tk.tile([2, NCLS], F32, tag="oo2")
                nc.vector.tensor_scalar_mul(oo[:], ex[:], rc[:, 0:1])
                nc.gpsimd.dma_start(o_fg_cls[v:v + 1, :], oo[0:1, :])
                nc.gpsimd.dma_start(o_bg_cls[v:v + 1, :], oo[1:2, :])

            # ---------------- emission order ----------------
            def s2k(v):
                pack = tk.tile([42, T], F32, tag="pack")
                stage2(v, pack)
                topk(v, pack)

            w_prep_ot(0)
            w_prep_ot(1)
            wts = {0: wT_load8(0, "wTa")}
            nc.gpsimd.dma_start(x_bf[0], xi[0])
            nc.gpsimd.dma_start(x_bf[1], xi[1])
            xts = {0: xT_load(0, "xTa")}
            conv8(0, 0, wts[0], xts[0])
            xts[1] = xT_load(1, "xTb")
            w_prep_ot(2)
            w_prep_ot(3)
            wts[1] = wT_load8(1, "wTb")
            conv8(0, 1, wts[0], xts[1])
            for h8 in range(1, 8):
                conv8(h8, 0, wts[h8], xts[0])
                if h8 < 7:
                    if 2 * h8 + 3 < NDT:
                        w_prep_ot(2 * h8 + 2)
                        w_prep_ot(2 * h8 + 3)
                    wts[h8 + 1] = wT_load8(
                        h8 + 1, "wTa" if (h8 + 1) % 2 == 0 else "wTb")
                    if h8 == 4:
                        nc.gpsimd.dma_start(x_bf[2], xi[2])
                    if h8 == 5:
                        nc.gpsimd.dma_start(x_bf[3], xi[3])
                    conv8(h8, 1, wts[h8], xts[1])
                else:
                    s2k(0)
                    xts[2] = xT_load(2, "xTa")
                    wts2 = {0: wT_load8(0, "wTa")}
                    conv8(7, 1, wts[7], xts[1])
                    s2k(1)
                    xts[3] = xT_load(3, "xTb")
            conv8(0, 2, wts2[0], xts[2])
            wts2[1] = wT_load8(1, "wTb")
            conv8(0, 3, wts2[0], xts[3])
            for h8 in range(1, 8):
                conv8(h8, 2, wts2[h8], xts[2])
                if h8 < 7:
                    wts2[h8 + 1] = wT_load8(
                        h8 + 1, "wTa" if (h8 + 1) % 2 == 0 else "wTb")
                    conv8(h8, 3, wts2[h8], xts[3])
                else:
                    s2k(2)
                    conv8(7, 3, wts2[7], xts[3])
                    s2k(3)

        for _rep in range(int(os.environ.get('BASS_NREP', '1'))):
            emit_once(_rep)

    nc.compile()
    return nc


_NC_CACHE = None


def _get_nc():
    global _NC_CACHE
    if _NC_CACHE is None:
        _NC_CACHE = build_nc()
    return _NC_CACHE


def make_in_maps(input_feature, conv_w, conv_b, att_w, att_b, cls_w, cls_b):
    input_feature = np.ascontiguousarray(input_feature, dtype=np.float32)
    conv_w = np.ascontiguousarray(conv_w, dtype=np.float32)
    conv_b = np.asarray(conv_b, dtype=np.float32)
    att_w = np.asarray(att_w, dtype=np.float32).reshape(2, D)
    att_b = np.asarray(att_b, dtype=np.float32)
    cls_w = np.asarray(cls_w, dtype=np.float32)
    cls_b = np.asarray(cls_b, dtype=np.float32)

    cmb = np.concatenate(
        [cls_w, att_w, (att_w[0] - att_w[1])[None, :]], axis=0)  # [24, D]
    cmbt = np.ascontiguousarray(cmb.T)  # [D, 24]
    zbias = np.concatenate(
        [cls_b, att_b, np.array([att_b[0] - att_b[1]], np.float32)]
    ).reshape(NCR, 1).astype(np.float32)
    cbias = np.ascontiguousarray(conv_b.reshape(NDT, 128).T)  # [128, 16]
    idm = np.eye(128, dtype=np.float32)
    kv = np.concatenate([np.full(NCLS, FGK), np.full(NCLS, BGK)]
                        ).reshape(42, 1).astype(np.float32)
    ki = np.concatenate([np.full(NCLS, 1.0 / FGK), np.full(NCLS, 1.0 / BGK)]
                        ).reshape(42, 1).astype(np.float32)

    in_maps = []
    for i in range(NCORES):
        in_maps.append({
            "x": np.ascontiguousarray(input_feature[i * BL:(i + 1) * BL]),
            "conv_w": conv_w,
            "cbias": cbias,
            "cmbt": cmbt,
            "zbias": zbias,
            "ident": idm,
            "kvec": kv,
            "kinv": ki,
        })
    return in_maps


def gather(rs):
    fg_cls = np.concatenate([r["fg_cls"] for r in rs], axis=0)
    bg_cls = np.concatenate([r["bg_cls"] for r in rs], axis=0)
    temp_att = np.concatenate([r["temp_att"] for r in rs], axis=0)
    cas_sm = np.concatenate([r["cas_sm"] for r in rs], axis=0)
    fg_sm = np.concatenate([r["fg_sm"] for r in rs], axis=0)
    bg_sm = np.concatenate([r["bg_sm"] for r in rs], axis=0)
    return (fg_cls, bg_cls, temp_att, cas_sm, fg_sm, bg_sm)


def kernel(input_feature, conv_w, conv_b, att_w, att_b, cls_w, cls_b):
    nc = _get_nc()
    in_maps = make_in_maps(input_feature, conv_w, conv_b, att_w, att_b,
                           cls_w, cls_b)
    res = run_bass_kernel_spmd(nc, in_maps, list(range(NCORES)))
    return gather(res.results)
